# revision 1
# baseline (speedup 1.0000x reference)
"""Trainium2 Bass kernel for nn_KMLoss (segment_reduce proto-network loss).

Math (exact decomposition of the reference):
  logit[q,s] = -0.5*sq(q,s), sq = |xq_q - xs_s|^2 (the reference's clamp at 0
  never fires for this data: min sq ~ 330).  Let L = 0.5*sq >= 0 (logit = -L).

  neg path: per-class column sums of logit are LINEAR in logit, so
    G[q,c] = sum_{s in c} L[q,s] = 0.5*cnt_c*|xq_q|^2 + 0.5*S2_c - xq_q . T_c
  with T_c = sum_{s in c} xs_s, S2_c = sum_{s in c} |xs_s|^2 -> tiny GEMM
  against per-class aggregates.  A = G/adj = -normalized (1/adj folded into
  host-prescaled GEMM columns; the query's own class is reordered to column 0
  so a single [128,1] tensor_scalar applies the self-correction).
  neg = LSE_c(normalized) = ln(sum exp(Mn2 - A)) - Mn2,  Mn2 = min(A).

  pos path: logsumexp over same-class support columns only -> block-diagonal
  [nq_c x ns_c] blocks once queries and support are grouped by class (~1/64
  of the full [Nq,Ns] logit matrix).  Self column pushed out via +2000 mask.

Sharding: core k owns classes [8k, 8k+8); its queries are the queries of
those classes (padded to 128 rows/class -> 8 uniform blocks per core, so the
SPMD program is identical across cores; only input data differs).  Per block
one matmul group computes [128, W+64] = [pos-block | A-block] in PSUM.
Device outputs per-row (min, sum-exp) for both paths; host takes logs and
means.  All DMAs are whole-tensor contiguous (8 loads + 1 store per core).
"""

import sys

import numpy as np

sys.path.insert(0, "/opt/trn_rl_repo")

NCORES = 8
C = 64
CPB = C // NCORES  # classes per core == blocks per core
QC = 128  # padded query rows per class
D = 256
KAUG = 2  # fp32 augmented contraction rows: [0.5*|xq|^2 ; 1]
INF = 1000.0

_PROGRAM_CACHE = {}


def _build_program(W):
    """Build the SPMD-uniform Bass program for class-block width W."""
    import concourse.bacc as bacc
    import concourse.tile as tile
    from concourse import mybir

    dt = mybir.dt
    Alu = mybir.AluOpType
    Act = mybir.ActivationFunctionType
    NCOL = W + C

    nc = bacc.Bacc(
        "TRN2",
        target_bir_lowering=False,
        debug=False,
        enable_asserts=False,
        num_devices=NCORES,
    )

    lhs0 = nc.dram_tensor("lhs0", [128, CPB * QC], dt.bfloat16, kind="ExternalInput").ap()
    lhs1 = nc.dram_tensor("lhs1", [128, CPB * QC], dt.bfloat16, kind="ExternalInput").ap()
    lhs2 = nc.dram_tensor("lhs2", [KAUG, CPB * QC], dt.float32, kind="ExternalInput").ap()
    rhs0 = nc.dram_tensor("rhs0", [128, CPB * NCOL], dt.bfloat16, kind="ExternalInput").ap()
    rhs1 = nc.dram_tensor("rhs1", [128, CPB * NCOL], dt.bfloat16, kind="ExternalInput").ap()
    rhs2 = nc.dram_tensor("rhs2", [KAUG, CPB * NCOL], dt.float32, kind="ExternalInput").ap()
    mask = nc.dram_tensor("mask", [128, CPB * W], dt.bfloat16, kind="ExternalInput").ap()
    corrdiv = nc.dram_tensor("corrdiv", [128, CPB], dt.float32, kind="ExternalInput").ap()
    out = nc.dram_tensor("out", [128, 4 * CPB], dt.float32, kind="ExternalOutput").ap()

    with tile.TileContext(nc) as tc:
        with (
            tc.tile_pool(name="io", bufs=1) as io,
            tc.tile_pool(name="work", bufs=3) as work,
            tc.tile_pool(name="pp", bufs=4, space="PSUM") as pp,
        ):
            s_l0 = io.tile([128, CPB * QC], dt.bfloat16)
            nc.sync.dma_start(out=s_l0, in_=lhs0)
            s_l1 = io.tile([128, CPB * QC], dt.bfloat16)
            nc.sync.dma_start(out=s_l1, in_=lhs1)
            s_l2 = io.tile([KAUG, CPB * QC], dt.float32)
            nc.sync.dma_start(out=s_l2, in_=lhs2)
            s_r0 = io.tile([128, CPB * NCOL], dt.bfloat16)
            nc.sync.dma_start(out=s_r0, in_=rhs0)
            s_r1 = io.tile([128, CPB * NCOL], dt.bfloat16)
            nc.sync.dma_start(out=s_r1, in_=rhs1)
            s_r2 = io.tile([KAUG, CPB * NCOL], dt.float32)
            nc.sync.dma_start(out=s_r2, in_=rhs2)
            s_mk = io.tile([128, CPB * W], dt.bfloat16)
            nc.sync.dma_start(out=s_mk, in_=mask)
            s_cd = io.tile([128, CPB], dt.float32)
            nc.sync.dma_start(out=s_cd, in_=corrdiv)

            # packed output: [Sn | Mn2 | S | Mn] each [128, CPB]
            outt = io.tile([128, 4 * CPB], dt.float32)
            Snall = outt[:, 0:CPB]
            Mn2all = outt[:, CPB:2 * CPB]
            Sall = outt[:, 2 * CPB:3 * CPB]
            Mnall = outt[:, 3 * CPB:4 * CPB]

            for b in range(CPB):
                qs = slice(b * QC, (b + 1) * QC)
                cs = slice(b * NCOL, (b + 1) * NCOL)
                ps = pp.tile([128, NCOL], dt.float32)
                nc.tensor.matmul(ps, s_l0[:, qs], s_r0[:, cs], start=True, stop=False)
                nc.tensor.matmul(ps, s_l1[:, qs], s_r1[:, cs], start=False, stop=False)
                nc.tensor.matmul(ps, s_l2[:, qs], s_r2[:, cs], start=False, stop=True)

                # neg path: own-class column (WG col 0) self-correction, then
                # Mn2 = min(A), Sn = sum exp(Mn2 - A)  over A = ps[:, W:]
                nc.vector.tensor_scalar_sub(
                    out=ps[:, W:W + 1], in0=ps[:, W:W + 1], scalar1=s_cd[:, b:b + 1]
                )
                nc.vector.tensor_reduce(
                    out=Mn2all[:, b:b + 1], in_=ps[:, W:NCOL],
                    axis=mybir.AxisListType.X, op=Alu.min,
                )
                En = work.tile([128, C], dt.float32, tag="En")
                nc.scalar.activation(
                    En, ps[:, W:NCOL], Act.Exp, bias=Mn2all[:, b:b + 1],
                    scale=-1.0, accum_out=Snall[:, b:b + 1],
                )

                # pos path: P2 = L + mask ; Mn = min ; S = sum exp(Mn - P2)
                P2 = work.tile([128, W], dt.float32, tag="P2")
                nc.vector.tensor_tensor(
                    out=P2, in0=ps[:, 0:W], in1=s_mk[:, b * W:(b + 1) * W], op=Alu.add
                )
                nc.vector.tensor_reduce(
                    out=Mnall[:, b:b + 1], in_=P2,
                    axis=mybir.AxisListType.X, op=Alu.min,
                )
                E = work.tile([128, W], dt.float32, tag="E")
                nc.scalar.activation(
                    E, P2, Act.Exp, bias=Mnall[:, b:b + 1], scale=-1.0,
                    accum_out=Sall[:, b:b + 1],
                )

            nc.sync.dma_start(out=out, in_=outt)

    nc.compile()
    return nc


def _prepare(xq, yq, xs, ys, pos):
    """Host-side prep: class grouping, aggregates, per-core input arrays."""
    import ml_dtypes

    bf16 = ml_dtypes.bfloat16
    Nq = xq.shape[0]
    xq64 = xq.astype(np.float64)
    xs64 = xs.astype(np.float64)

    cnt = np.bincount(ys, minlength=C).astype(np.float64)
    T_c = np.zeros((C, D), np.float64)
    np.add.at(T_c, ys, xs64)
    S2_c = np.zeros(C, np.float64)
    np.add.at(S2_c, ys, (xs64 ** 2).sum(-1))
    S2_eff = np.where(cnt > 0, S2_c, 4e6)  # empty class -> huge A -> excluded

    xq2 = (xq64 ** 2).sum(-1)
    xs2 = (xs64 ** 2).sum(-1)

    sidx = [np.where(ys == c)[0] for c in range(C)]
    qidx = [np.where(yq == c)[0] for c in range(C)]
    max_ns = max(1, max(len(s) for s in sidx))
    max_nq = max(len(q) for q in qidx)
    assert max_nq <= QC, f"class query count {max_nq} exceeds {QC}"
    W = -(-max_ns // 16) * 16
    NCOL = W + C

    xs_twin = xs64[pos]
    L_self = 0.5 * ((xq64 - xs_twin) ** 2).sum(-1)

    in_maps = []
    meta = []
    for k in range(NCORES):
        lhs_dot = np.zeros((D, CPB * QC), np.float32)
        lhs_aug = np.zeros((KAUG, CPB * QC), np.float32)
        rhs_dot = np.zeros((D, CPB * NCOL), np.float32)
        rhs_aug = np.zeros((KAUG, CPB * NCOL), np.float32)
        mk = np.zeros((128, CPB * W), np.float32)
        cd = np.zeros((128, CPB), np.float32)
        core_meta = []
        for b in range(CPB):
            cb = k * CPB + b
            qi = qidx[cb]
            si = sidx[cb]
            nq, ns = len(qi), len(si)
            qs = slice(b * QC, b * QC + nq)
            lhs_dot[:, qs] = -xq[qi].T
            lhs_aug[0, qs] = (0.5 * xq2[qi]).astype(np.float32)
            lhs_aug[1, qs] = 1.0
            # pos columns
            ss = slice(b * NCOL, b * NCOL + ns)
            rhs_dot[:, ss] = xs[si].T
            rhs_aug[0, ss] = 1.0
            rhs_aug[1, ss] = (0.5 * xs2[si]).astype(np.float32)
            # pad pos columns: L_pad = 0.5*xq2 + 2000
            ps_ = slice(b * NCOL + ns, b * NCOL + W)
            rhs_aug[0, ps_] = 1.0
            rhs_aug[1, ps_] = 2000.0
            # WG columns, own class first, scaled by 1/adj
            order = [cb] + [c for c in range(C) if c != cb]
            adj = np.array(
                [cnt[c] - (1.0 if c == cb else 0.0) for c in order], np.float64
            )
            s_j = 1.0 / np.maximum(adj, 1.0)
            s_j[adj <= 0] = 1.0
            ocols = np.array(order)
            gs = slice(b * NCOL + W, (b + 1) * NCOL)
            rhs_dot[:, gs] = (T_c[ocols].T * s_j[None, :]).astype(np.float32)
            rhs_aug[0, gs] = (cnt[ocols] * s_j).astype(np.float32)
            rhs_aug[1, gs] = (0.5 * S2_eff[ocols] * s_j).astype(np.float32)
            if nq:
                selfpos = np.searchsorted(si, pos[qi])
                assert ns and (si[selfpos] == pos[qi]).all(), \
                    "pos[q] must be same-class support"
                r = np.arange(nq)
                single = cnt[cb] <= 1
                mk[r, b * W + selfpos] = np.where(
                    single, -L_self[qi], 2000.0
                ).astype(np.float32)
                corr = L_self[qi] - INF * (cnt[cb] > 1)
                cd[:nq, b] = (corr * s_j[0]).astype(np.float32)
            core_meta.append((cb, qi))
        in_maps.append({
            "lhs0": lhs_dot[0:128].astype(bf16),
            "lhs1": lhs_dot[128:256].astype(bf16),
            "lhs2": lhs_aug,
            "rhs0": rhs_dot[0:128].astype(bf16),
            "rhs1": rhs_dot[128:256].astype(bf16),
            "rhs2": rhs_aug,
            "mask": mk.astype(bf16),
            "corrdiv": cd,
        })
        meta.append(core_meta)
    return W, in_maps, meta, Nq


def _reduce_host(results, meta, Nq):
    total = 0.0
    for k in range(NCORES):
        o = np.asarray(results[k]["out"], np.float64)
        Sn, Mn2 = o[:, 0:CPB], o[:, CPB:2 * CPB]
        S, Mn = o[:, 2 * CPB:3 * CPB], o[:, 3 * CPB:4 * CPB]
        neg = np.log(Sn) - Mn2
        pos = np.log(S) - Mn
        for b, (cb, qi) in enumerate(meta[k]):
            n = len(qi)
            if n:
                total += (neg[:n, b] - pos[:n, b]).sum()
    return np.array(total / Nq, dtype=np.float32)


def _run(xq, yq, xs, ys, pos, trace=False, tmpdir=None):
    from concourse import bass_utils

    xq = np.ascontiguousarray(np.asarray(xq, np.float32))
    xs = np.ascontiguousarray(np.asarray(xs, np.float32))
    yq = np.asarray(yq).astype(np.int64)
    ys = np.asarray(ys).astype(np.int64)
    pos = np.asarray(pos).astype(np.int64)

    W, in_maps, meta, Nq = _prepare(xq, yq, xs, ys, pos)
    if W not in _PROGRAM_CACHE:
        _PROGRAM_CACHE[W] = _build_program(W)
    nc = _PROGRAM_CACHE[W]

    kw = {}
    if trace:
        kw = dict(trace=True, tmpdir=tmpdir)
    res = bass_utils.run_bass_kernel_spmd(
        nc, in_maps, core_ids=list(range(NCORES)), **kw
    )
    return _reduce_host(res.results, meta, Nq), res


def kernel(xq, yq, xs, ys, pos):
    loss, _ = _run(xq, yq, xs, ys, pos, trace=False)
    return loss



# revision 3
# speedup vs baseline: 1.3606x; 1.3606x over previous
"""Trainium2 Bass kernel for nn_KMLoss (segment_reduce proto-network loss).

Math (exact decomposition of the reference; see reference for notation):
  L[q,s] = 0.5*|xq_q - xs_s|^2, logit = -L.  All device values drop the
  per-query term 0.5|xq|^2 (it cancels in neg-pos; host re-adds where needed)
  and are shifted by global constants so fp32 exp never over/underflows:
    pos path value:  P[q,j] = (0.5|xs_j|^2 - SHP) - xq.xs_j     (same-class j)
    neg path value:  A[q,c] = (0.5*S2_c/cnt_c - SA) - xq.(T_c/cnt_c)
  with T_c/S2_c the per-class sum / sum-of-squares of xs.  The own-class
  column of A gets a per-query correction `fix` (removes the self support
  entry and applies the -INF mask + cnt-1 denominator of the reference),
  folded into the matmul as an extra contraction row times an indicator
  column.  Device computes E = exp(-V) for both paths and returns per-row
  sums only; host takes logs, subtracts the (host-simulated, fp8-identical)
  self column from the pos sum, and averages.

Per core: 8 class-blocks (core k owns the classes ranked {b*8+k} by size).
Block b accumulates PSUM bank b via: one K=4 bf16 aug matmul (column
constants + fix row; 4-way row-tiled across blocks, runs first with
start=True while input DMAs stream) + two K=128 fp8 dot matmuls.  Chunked
exp (ACT) + sum-reduce (DVE) ops follow per quad/pair/block, sized so the
pipeline tail stays short.  8 input DMAs + 2 output DMAs per core.
"""

import sys

import numpy as np

sys.path.insert(0, "/opt/trn_rl_repo")

NCORES = 8
C = 64
CPB = C // NCORES  # class-blocks per core
D = 256
INF = 1000.0

# chunking of the 8 blocks for the exp/reduce stage: quad, pair, single, single
CHUNKS = [(0, 4), (4, 6), (6, 7), (7, 8)]

_PROGRAM_CACHE = {}


def _round_up(x, m):
    return -(-x // m) * m


def _build_program(QC, WS):
    """SPMD-uniform Bass program. QC = padded queries/block, WS = per-block
    support widths (uniform within each chunk of CHUNKS)."""
    import concourse.bacc as bacc
    import concourse.tile as tile
    from concourse import mybir

    dt = mybir.dt
    Alu = mybir.AluOpType
    Act = mybir.ActivationFunctionType
    f8 = dt.float8e4
    NC = [w + C for w in WS]  # per-block bank width (pos + agg cols)

    nc = bacc.Bacc(
        "TRN2",
        target_bir_lowering=False,
        debug=False,
        enable_asserts=False,
        num_devices=NCORES,
    )

    # aug: 4 row-groups g, each [4, 2*QC + NC[g] + NC[4+g]] packed side by side
    aug_cols = [2 * QC + NC[g] + NC[4 + g] for g in range(4)]
    aug_d = nc.dram_tensor("aug", [4, sum(aug_cols)], dt.bfloat16, kind="ExternalInput").ap()
    # pair p: [128, 4*QC + 2*NC[2p] + 2*NC[2p+1]] fp8 (lhs halves then rhs halves)
    pair_cols = [4 * QC + 2 * NC[2 * p] + 2 * NC[2 * p + 1] for p in range(4)]
    pair_d = [
        nc.dram_tensor(f"pair{p}", [128, pair_cols[p]], f8, kind="ExternalInput").ap()
        for p in range(4)
    ]
    out_d = nc.dram_tensor("out", [QC, 16], dt.float32, kind="ExternalOutput").ap()

    with tile.TileContext(nc) as tc:
        with (
            tc.tile_pool(name="io", bufs=1) as io,
            tc.tile_pool(name="pp", bufs=1, space="PSUM") as pp,
        ):
            warm_i = io.tile([128, 16], dt.float32)
            warm_o = io.tile([128, 16], dt.float32)
            t_aug = io.tile([128, max(aug_cols)], dt.bfloat16)
            t_pair = [io.tile([128, pair_cols[p]], f8, name=f"t_pair{p}") for p in range(4)]
            e_t = [io.tile([128, (hi - lo) * NC[lo]], dt.float32, name=f"e_t{lo}")
                   for lo, hi in CHUNKS]
            out0 = io.tile([128, 8], dt.float32)
            out1 = io.tile([128, 8], dt.float32)
            ps0 = pp.tile([128, 2048], dt.float32)
            ps1 = pp.tile([128, 2048], dt.float32)

            # ACT table warm-up (loads the exp table set during the DMAs)
            nc.vector.memset(warm_i[:, :], 0.0)
            nc.scalar.activation(warm_o[:, :], warm_i[:, :], Act.Exp)

            # input DMAs: aug row-groups first (aug matmuls run first), then pairs
            off = 0
            for g in range(4):
                nc.sync.dma_start(
                    out=t_aug[32 * g:32 * g + 4, 0:aug_cols[g]],
                    in_=aug_d[0:4, off:off + aug_cols[g]],
                )
                off += aug_cols[g]
            for p in range(4):
                nc.sync.dma_start(out=t_pair[p], in_=pair_d[p])

            def bank(b):
                ps = ps0 if b < 4 else ps1
                return ps[0:QC, (b % 4) * 512:(b % 4) * 512 + NC[b]]

            # aug matmuls: start each bank's accumulation group (4-way row tiling)
            for b in range(8):
                g = b % 4
                la = 0 if b < 4 else QC
                ra = 2 * QC + (0 if b < 4 else NC[g])
                nc.tensor.matmul(
                    bank(b),
                    t_aug[32 * g:32 * g + 4, la:la + QC],
                    t_aug[32 * g:32 * g + 4, ra:ra + NC[b]],
                    start=True,
                    stop=False,
                    tile_position=(32 * g, 0),
                )

            # dot matmuls + per-chunk exp/sum; emit chunk ops right after the
            # dots they depend on so the scheduler pipelines ACT/DVE behind PE
            def emit_dots(b):
                p, s = b // 2, b % 2
                rbase = 4 * QC + s * 2 * NC[b]
                for h in range(2):
                    nc.tensor.matmul(
                        bank(b),
                        t_pair[p][:, s * 176 + h * QC:s * 176 + (h + 1) * QC],
                        t_pair[p][:, rbase + h * NC[b]:rbase + (h + 1) * NC[b]],
                        start=False,
                        stop=(h == 1),
                    )

            def emit_chunk(ci):
                lo, hi = CHUNKS[ci]
                n = hi - lo
                W, NCb = WS[lo], NC[lo]
                ps = ps0 if lo < 4 else ps1
                base = (lo % 4) * 512
                if n > 1:
                    src = ps[0:QC, base:base + n * 512].rearrange(
                        "p (g c) -> p g c", g=n)[:, :, 0:NCb]
                    dst = e_t[ci][0:QC, :].rearrange("p (g c) -> p g c", c=NCb)
                else:
                    src = ps[0:QC, base:base + NCb]
                    dst = e_t[ci][0:QC, :]
                nc.scalar.activation(dst, src, Act.Exp, scale=-1.0)
                ev = e_t[ci][0:QC, :].rearrange("p (g c) -> p g c", c=NCb)
                ot = out0 if ci == 0 else out1
                oc = 0 if ci == 0 else lo - 4
                nc.vector.tensor_reduce(
                    out=ot[0:QC, oc:oc + n], in_=ev[:, :, 0:W],
                    axis=mybir.AxisListType.X, op=Alu.add,
                )
                nc.vector.tensor_reduce(
                    out=ot[0:QC, 4 + oc:4 + oc + n], in_=ev[:, :, W:NCb],
                    axis=mybir.AxisListType.X, op=Alu.add,
                )

            for b in range(4):
                emit_dots(b)
            emit_chunk(0)
            nc.sync.dma_start(out=out_d[:, 0:8], in_=out0[0:QC, :])
            for b in range(4, 6):
                emit_dots(b)
            emit_chunk(1)
            emit_dots(6)
            emit_chunk(2)
            emit_dots(7)
            emit_chunk(3)
            nc.sync.dma_start(out=out_d[:, 8:16], in_=out1[0:QC, :])

    nc.compile()
    return nc


def _prepare(xq, yq, xs, ys, pos):
    """Host-side prep: class stats, shifts, per-core packed arrays."""
    import ml_dtypes

    bf16 = ml_dtypes.bfloat16
    f8 = ml_dtypes.float8_e4m3
    Nq = xq.shape[0]
    xq64 = xq.astype(np.float64)
    xs64 = xs.astype(np.float64)

    cnt = np.bincount(ys, minlength=C).astype(np.float64)
    assert (cnt > 1).all(), "singleton/empty classes unsupported by this kernel"
    T_c = np.zeros((C, D))
    np.add.at(T_c, ys, xs64)
    S2_c = np.zeros(C)
    np.add.at(S2_c, ys, (xs64 ** 2).sum(-1))
    xs2h = 0.5 * (xs64 ** 2).sum(-1)
    xq2h = 0.5 * (xq64 ** 2).sum(-1)
    agg_scaled = T_c / cnt[:, None]

    # global shifts with exact host guards
    A_exact = 0.5 * S2_c[None, :] / cnt[None, :] - xq.astype(np.float32) @ \
        agg_scaled.T.astype(np.float32)
    SA = float(np.round(np.median(A_exact)))
    assert np.abs(A_exact - SA).max() < 70, "neg-path shift margin exceeded"

    sidx = [np.where(ys == c)[0] for c in range(C)]
    qidx = [np.where(yq == c)[0] for c in range(C)]
    xsf = xs.astype(np.float32)
    xqf = xq.astype(np.float32)
    pm_lo, pm_hi = 1e30, -1e30
    for c in range(C):
        if len(qidx[c]) == 0 or len(sidx[c]) == 0:
            continue
        P = xs2h[sidx[c]][None, :].astype(np.float32) - xqf[qidx[c]] @ xsf[sidx[c]].T
        m = P.min(axis=1)
        pm_lo = min(pm_lo, float(m.min()))
        pm_hi = max(pm_hi, float(m.max()))
    assert pm_hi - pm_lo < 150, "pos-path shift window too wide"
    SHP = float(np.round((pm_lo + pm_hi) / 2))

    # per-query own-class correction
    H = 0.5 * S2_c[yq] - np.einsum('qd,qd->q', xq64, T_c[yq])
    Lself_t = xs2h[pos] - np.einsum('qd,qd->q', xq64, xs64[pos])
    adj = cnt[yq] - 1.0
    fix = xq2h / adj + H / (cnt[yq] * adj) + (INF - (Lself_t + xq2h)) / adj

    def hilo(v):
        hi = v.astype(bf16).astype(np.float64)
        lo = (v - hi).astype(bf16)
        return hi.astype(bf16), lo

    fix_hi, fix_lo = hilo(fix)
    cpos_hi, cpos_lo = hilo(xs2h - SHP)
    cagg_hi, cagg_lo = hilo(0.5 * S2_c / cnt - SA)
    cpos_dev = cpos_hi.astype(np.float64) + cpos_lo.astype(np.float64)

    # class -> (core, block) by descending support count; chunk-uniform widths
    order = np.argsort(-cnt, kind='stable')
    QC = _round_up(max(max(len(q) for q in qidx), 1), 8)
    WS = [0] * 8
    for b in range(8):
        WS[b] = _round_up(max(len(sidx[order[b * 8 + k]]) for k in range(NCORES)), 16)
    for lo, hi in CHUNKS:
        w = max(WS[lo:hi])
        for b in range(lo, hi):
            WS[b] = w
    NC = [w + C for w in WS]
    assert max(NC) <= 512 and QC <= 128

    # fp8-rounded data (also used host-side for the self-column subtraction)
    xq_r8 = (-xqf).astype(f8)
    xs_r8 = xsf.astype(f8)
    agg_r8 = agg_scaled.astype(np.float32).astype(f8)

    aug_cols = [2 * QC + NC[g] + NC[4 + g] for g in range(4)]
    pair_cols = [4 * QC + 2 * NC[2 * p] + 2 * NC[2 * p + 1] for p in range(4)]

    in_maps = []
    meta = []
    for k in range(NCORES):
        aug = np.zeros((4, sum(aug_cols)), np.float32)
        pairs = [np.zeros((128, pair_cols[p]), f8) for p in range(4)]
        core_meta = []
        aug_off = [0] * 8
        off = 0
        for g in range(4):
            aug_off[g] = off
            aug_off[4 + g] = off  # same segment, second halves
            off += aug_cols[g]
        for b in range(8):
            cls = int(order[b * 8 + k])
            qi = qidx[cls]
            si = sidx[cls]
            nq, ns = len(qi), len(si)
            W, NCb = WS[b], NC[b]
            g = b % 4
            seg = aug_off[b]
            la = seg + (0 if b < 4 else QC)
            ra = seg + 2 * QC + (0 if b < 4 else NC[g])
            # aug lhs rows: [1, 1, fix_hi, fix_lo] over the block's queries
            aug[0, la:la + nq] = 1.0
            aug[1, la:la + nq] = 1.0
            aug[2, la:la + nq] = fix_hi[qi].astype(np.float32)
            aug[3, la:la + nq] = fix_lo[qi].astype(np.float32)
            # aug rhs rows: [c_hi, c_lo, ind, ind]
            aug[0, ra:ra + ns] = cpos_hi[si].astype(np.float32)
            aug[1, ra:ra + ns] = cpos_lo[si].astype(np.float32)
            aug[0, ra + ns:ra + W] = 2000.0
            aug[0, ra + W:ra + NCb] = cagg_hi.astype(np.float32)
            aug[1, ra + W:ra + NCb] = cagg_lo.astype(np.float32)
            aug[2, ra + W + cls] = 1.0
            aug[3, ra + W + cls] = 1.0
            # pair tile: lhs halves then rhs halves
            p, s = b // 2, b % 2
            P = pairs[p]
            for h in range(2):
                rows = slice(h * 128, (h + 1) * 128)
                P[:, s * 176 + h * QC:s * 176 + h * QC + nq] = xq_r8[qi].T[rows]
                rb = 4 * QC + s * 2 * NCb + h * NCb
                P[:, rb:rb + ns] = xs_r8[si].T[rows]
                P[:, rb + W:rb + NCb] = agg_r8.T[rows]
            core_meta.append((cls, qi))
        im = {"aug": aug.astype(bf16)}
        for p in range(4):
            im[f"pair{p}"] = pairs[p]
        in_maps.append(im)
        meta.append(core_meta)

    host = {
        "SHP": SHP, "SA": SA, "xq2h": xq2h,
        "tdev": np.exp(-((np.einsum('qd,qd->q',
                                    xq_r8.astype(np.float32),
                                    xs_r8[pos].astype(np.float32))
                          ).astype(np.float32).astype(np.float64)
                         + cpos_dev[pos])),
    }
    return QC, WS, in_maps, meta, host, Nq


def _reduce_host(results, meta, host, Nq):
    SHP, SA = host["SHP"], host["SA"]
    total = 0.0
    for k in range(NCORES):
        o = np.asarray(results[k]["out"], np.float64)
        for b, (cls, qi) in enumerate(meta[k]):
            n = len(qi)
            if n == 0:
                continue
            col = b if b < 4 else 8 + (b - 4)
            S = o[0:n, col]
            Sn = o[0:n, col + 4]
            S_excl = np.maximum(S - host["tdev"][qi], S * 1e-7)
            pos_v = np.log(S_excl) - SHP
            neg_v = np.log(Sn) - SA
            total += (neg_v - pos_v).sum()
    return np.array(total / Nq, dtype=np.float32)


def _run(xq, yq, xs, ys, pos, trace=False, tmpdir=None):
    from concourse import bass_utils

    xq = np.ascontiguousarray(np.asarray(xq, np.float32))
    xs = np.ascontiguousarray(np.asarray(xs, np.float32))
    yq = np.asarray(yq).astype(np.int64)
    ys = np.asarray(ys).astype(np.int64)
    pos = np.asarray(pos).astype(np.int64)

    QC, WS, in_maps, meta, host, Nq = _prepare(xq, yq, xs, ys, pos)
    key = (QC, tuple(WS))
    if key not in _PROGRAM_CACHE:
        _PROGRAM_CACHE[key] = _build_program(QC, WS)
    nc = _PROGRAM_CACHE[key]

    kw = {}
    if trace:
        kw = dict(trace=True, tmpdir=tmpdir)
    res = bass_utils.run_bass_kernel_spmd(
        nc, in_maps, core_ids=list(range(NCORES)), **kw
    )
    return _reduce_host(res.results, meta, host, Nq), res


def kernel(xq, yq, xs, ys, pos):
    loss, _ = _run(xq, yq, xs, ys, pos, trace=False)
    return loss


# revision 5
# speedup vs baseline: 1.6209x; 1.1913x over previous
"""Trainium2 Bass kernel for nn_KMLoss (segment_reduce proto-network loss).

Math (exact decomposition of the reference):
  L[q,s] = 0.5*|xq_q - xs_s|^2, logit = -L.  All device values drop the
  per-query term 0.5|xq|^2 (it cancels in neg-pos) and are shifted by global
  constants so fp32 exp never over/underflows:
    pos path:  P[q,j] = (0.5|xs_j|^2 - SHP) - xq.xs_j      (same-class j)
    neg path:  A[q,c] = (0.5*S2_c/cnt_c - SA) - xq.(T_c/cnt_c)
  The own-class column of A gets a per-query correction `fix` (removes the
  self entry and applies the -INF mask + cnt-1 denominator), folded into the
  matmul as an extra contraction row times an indicator column.  The device
  returns per-row exp-sums only; host takes logs, subtracts the (fp8-identical
  host-simulated) self column from the pos sum, and averages.

Per core: 8 class-blocks (core k owns classes ranked {b*8+k} by size).  Block
b owns PSUM bank b: a K=4 bf16 aug matmul (column constants + fix row; 4-way
row-tiled, start=True, runs while inputs stream) + two K=128 fp8 dot matmuls.
3 input DMAs (aug + two merged pair loads, one issued from the ACT ring to
parallelize HWDGE descriptor generation), dummy matmuls to lift the PE HAM
throttle during the load, chunked exp (ACT) + sum-reduce (DVE) sized to keep
the tail short (last block's sums via the ACT accumulator), 2 output DMAs.
"""

import sys

import numpy as np

sys.path.insert(0, "/opt/trn_rl_repo")

NCORES = 8
C = 64
CPB = C // NCORES
D = 256
INF = 1000.0

# exp/reduce chunking of the 8 blocks; last chunk's sums use ACT accum
CHUNKS = [(0, 2), (2, 4), (4, 6), (6, 7), (7, 8)]
PS_BANKS = [2, 2, 2, 1, 1]  # PSUM banks per chunk (sum = 8)
N_WARM_MM = 8

_PROGRAM_CACHE = {}


def _round_up(x, m):
    return -(-x // m) * m


def _build_program(QC, WS):
    import concourse.bacc as bacc
    import concourse.tile as tile
    from concourse import mybir

    dt = mybir.dt
    Alu = mybir.AluOpType
    Act = mybir.ActivationFunctionType
    f8 = dt.float8e4
    NC = [w + C for w in WS]

    nc = bacc.Bacc(
        "TRN2",
        target_bir_lowering=False,
        debug=False,
        enable_asserts=False,
        num_devices=NCORES,
    )

    # aug: row-group g (partitions 32g..32g+3) holds blocks {g, 4+g}:
    # [lhs_g | lhs_4+g | rhs_g | rhs_4+g], zero-padded to uniform AUGW cols
    aug_cols = [2 * QC + NC[g] + NC[4 + g] for g in range(4)]
    AUGW = max(aug_cols)
    aug_d = nc.dram_tensor("aug", [100, AUGW], dt.bfloat16, kind="ExternalInput").ap()
    pair_cols = [4 * QC + 2 * NC[2 * p] + 2 * NC[2 * p + 1] for p in range(4)]
    in01_d = nc.dram_tensor("in01", [128, pair_cols[0] + pair_cols[1]], f8,
                            kind="ExternalInput").ap()
    in23_d = nc.dram_tensor("in23", [128, pair_cols[2] + pair_cols[3]], f8,
                            kind="ExternalInput").ap()
    out_d = nc.dram_tensor("out", [QC, 16], dt.float32, kind="ExternalOutput").ap()

    with tile.TileContext(nc) as tc:
        with (
            tc.tile_pool(name="io", bufs=1) as io,
            tc.tile_pool(name="pp", bufs=1, space="PSUM") as pp,
        ):
            warm_i = io.tile([128, 16], dt.float32)
            warm_o = io.tile([128, 16], dt.float32)
            w_s = io.tile([128, 128], dt.bfloat16)
            t_aug = io.tile([128, AUGW], dt.bfloat16)
            t_in01 = io.tile([128, pair_cols[0] + pair_cols[1]], f8)
            t_in23 = io.tile([128, pair_cols[2] + pair_cols[3]], f8)
            e_t = [io.tile([128, (hi - lo) * NC[lo]], dt.float32, name=f"e_t{lo}")
                   for lo, hi in CHUNKS]
            out0 = io.tile([128, 8], dt.float32)
            out1 = io.tile([128, 8], dt.float32)
            ps_t = [pp.tile([128, 512 * nb], dt.float32, name=f"ps_c{i}")
                    for i, nb in enumerate(PS_BANKS)]

            # scratch init on gpsimd (keeps DVE free); exp-table warm on ACT
            nc.gpsimd.memset(warm_i[:, :], 0.0)
            nc.gpsimd.memset(w_s[:, :], 0.0)
            nc.scalar.activation(warm_o[:, :], warm_i[:, :], Act.Exp)

            # input DMAs: aug + in01 on the sync HWDGE ring, in23 on the ACT
            # ring (descriptor generation runs in parallel with sync's)
            nc.sync.dma_start(out=t_aug[0:100, :], in_=aug_d)
            nc.sync.dma_start(out=t_in01, in_=in01_d)
            nc.scalar.dma_start(out=t_in23, in_=in23_d)

            # dummy matmuls: lift the PE HAM clock gate while inputs stream
            for _ in range(N_WARM_MM):
                nc.tensor.matmul(
                    ps_t[0][0:128, 0:128], w_s[:, 0:128], w_s[:, 0:128],
                    start=True, stop=True,
                )

            def chunk_of(b):
                for ci, (lo, hi) in enumerate(CHUNKS):
                    if lo <= b < hi:
                        return ci, b - lo
                raise AssertionError

            def bank(b):
                ci, off = chunk_of(b)
                return ps_t[ci][0:QC, off * 512:off * 512 + NC[b]]

            # aug matmuls open each bank's accumulation group (4-way row tiled)
            for b in range(8):
                g = b % 4
                la = (0 if b < 4 else QC)
                ra = 2 * QC + (0 if b < 4 else NC[g])
                nc.tensor.matmul(
                    bank(b),
                    t_aug[32 * g:32 * g + 4, la:la + QC],
                    t_aug[32 * g:32 * g + 4, ra:ra + NC[b]],
                    start=True,
                    stop=False,
                    tile_position=(32 * g, 0),
                )

            def emit_dots(b):
                p, s = b // 2, b % 2
                t_in = t_in01 if b < 4 else t_in23
                pb = sum(pair_cols[2 * (p // 2):p])  # offset of pair p in its tensor
                rbase = pb + 4 * QC + s * 2 * NC[b]
                for h in range(2):
                    nc.tensor.matmul(
                        bank(b),
                        t_in[:, pb + s * 2 * QC + h * QC:pb + s * 2 * QC + (h + 1) * QC],
                        t_in[:, rbase + h * NC[b]:rbase + (h + 1) * NC[b]],
                        start=False,
                        stop=(h == 1),
                    )

            def emit_chunk(ci):
                lo, hi = CHUNKS[ci]
                n = hi - lo
                W, NCb = WS[lo], NC[lo]
                ps = ps_t[ci]
                if n > 1:
                    src = ps[0:QC, 0:n * 512].rearrange(
                        "p (g c) -> p g c", g=n)[:, :, 0:NCb]
                    dst = e_t[ci][0:QC, :].rearrange("p (g c) -> p g c", c=NCb)
                else:
                    src = ps[0:QC, 0:NCb]
                    dst = e_t[ci][0:QC, :]
                ot = out0 if lo < 4 else out1
                oc = lo % 4
                if ci == len(CHUNKS) - 1:
                    # single block: exp pos / agg separately, sums via ACT accum
                    nc.scalar.activation(
                        e_t[ci][0:QC, 0:W], ps[0:QC, 0:W], Act.Exp, scale=-1.0,
                        accum_out=ot[0:QC, oc:oc + 1],
                    )
                    nc.scalar.activation(
                        e_t[ci][0:QC, W:NCb], ps[0:QC, W:NCb], Act.Exp, scale=-1.0,
                        accum_out=ot[0:QC, 4 + oc:4 + oc + 1],
                    )
                    return
                nc.scalar.activation(dst, src, Act.Exp, scale=-1.0)
                ev = e_t[ci][0:QC, :].rearrange("p (g c) -> p g c", c=NCb)
                nc.vector.tensor_reduce(
                    out=ot[0:QC, oc:oc + n], in_=ev[:, :, 0:W],
                    axis=mybir.AxisListType.X, op=Alu.add,
                )
                nc.vector.tensor_reduce(
                    out=ot[0:QC, 4 + oc:4 + oc + n], in_=ev[:, :, W:NCb],
                    axis=mybir.AxisListType.X, op=Alu.add,
                )

            for b in range(2):
                emit_dots(b)
            emit_chunk(0)
            for b in range(2, 4):
                emit_dots(b)
            emit_chunk(1)
            nc.sync.dma_start(out=out_d[:, 0:8], in_=out0[0:QC, :])
            for b in range(4, 6):
                emit_dots(b)
            emit_chunk(2)
            emit_dots(6)
            emit_chunk(3)
            emit_dots(7)
            emit_chunk(4)
            nc.sync.dma_start(out=out_d[:, 8:16], in_=out1[0:QC, :])

    nc.compile()
    return nc


def _prepare(xq, yq, xs, ys, pos):
    """Host-side prep: class stats, shifts, per-core packed arrays."""
    import ml_dtypes

    bf16 = ml_dtypes.bfloat16
    f8 = ml_dtypes.float8_e4m3
    Nq = xq.shape[0]
    xq64 = xq.astype(np.float64)
    xs64 = xs.astype(np.float64)

    cnt = np.bincount(ys, minlength=C).astype(np.float64)
    assert (cnt > 1).all(), "singleton/empty classes unsupported by this kernel"
    T_c = np.zeros((C, D))
    np.add.at(T_c, ys, xs64)
    S2_c = np.zeros(C)
    np.add.at(S2_c, ys, (xs64 ** 2).sum(-1))
    xs2h = 0.5 * (xs64 ** 2).sum(-1)
    xq2h = 0.5 * (xq64 ** 2).sum(-1)
    agg_scaled = T_c / cnt[:, None]

    A_exact = 0.5 * S2_c[None, :] / cnt[None, :] - xq.astype(np.float32) @ \
        agg_scaled.T.astype(np.float32)
    SA = float(np.round(np.median(A_exact)))
    assert np.abs(A_exact - SA).max() < 70, "neg-path shift margin exceeded"

    sidx = [np.where(ys == c)[0] for c in range(C)]
    qidx = [np.where(yq == c)[0] for c in range(C)]
    xsf = xs.astype(np.float32)
    xqf = xq.astype(np.float32)
    pm_lo, pm_hi = 1e30, -1e30
    for c in range(C):
        if len(qidx[c]) == 0 or len(sidx[c]) == 0:
            continue
        P = xs2h[sidx[c]][None, :].astype(np.float32) - xqf[qidx[c]] @ xsf[sidx[c]].T
        m = P.min(axis=1)
        pm_lo = min(pm_lo, float(m.min()))
        pm_hi = max(pm_hi, float(m.max()))
    assert pm_hi - pm_lo < 150, "pos-path shift window too wide"
    SHP = float(np.round((pm_lo + pm_hi) / 2))

    H = 0.5 * S2_c[yq] - np.einsum('qd,qd->q', xq64, T_c[yq])
    Lself_t = xs2h[pos] - np.einsum('qd,qd->q', xq64, xs64[pos])
    adj = cnt[yq] - 1.0
    fix = xq2h / adj + H / (cnt[yq] * adj) + (INF - (Lself_t + xq2h)) / adj

    def hilo(v):
        hi = v.astype(bf16).astype(np.float64)
        lo = (v - hi).astype(bf16)
        return hi.astype(bf16), lo

    fix_hi, fix_lo = hilo(fix)
    cpos_hi, cpos_lo = hilo(xs2h - SHP)
    cagg_hi, cagg_lo = hilo(0.5 * S2_c / cnt - SA)
    cpos_dev = cpos_hi.astype(np.float64) + cpos_lo.astype(np.float64)

    order = np.argsort(-cnt, kind='stable')
    QC = _round_up(max(max(len(q) for q in qidx), 1), 8)
    WS = [0] * 8
    for b in range(8):
        WS[b] = _round_up(max(len(sidx[order[b * 8 + k]]) for k in range(NCORES)), 16)
    for lo, hi in CHUNKS:
        w = max(WS[lo:hi])
        for b in range(lo, hi):
            WS[b] = w
    NC = [w + C for w in WS]
    assert max(NC) <= 512 and QC <= 128

    xq_r8 = (-xqf).astype(f8)
    xs_r8 = xsf.astype(f8)
    agg_r8 = agg_scaled.astype(np.float32).astype(f8)

    aug_cols = [2 * QC + NC[g] + NC[4 + g] for g in range(4)]
    AUGW = max(aug_cols)
    pair_cols = [4 * QC + 2 * NC[2 * p] + 2 * NC[2 * p + 1] for p in range(4)]

    in_maps = []
    meta = []
    for k in range(NCORES):
        aug = np.zeros((100, AUGW), np.float32)
        pairs = [np.zeros((128, pair_cols[p]), f8) for p in range(4)]
        core_meta = []
        for b in range(8):
            cls = int(order[b * 8 + k])
            qi = qidx[cls]
            si = sidx[cls]
            nq, ns = len(qi), len(si)
            W, NCb = WS[b], NC[b]
            g = b % 4
            la = (0 if b < 4 else QC)
            ra = 2 * QC + (0 if b < 4 else NC[g])
            gp = 32 * g
            aug[gp + 0, la:la + nq] = 1.0
            aug[gp + 1, la:la + nq] = 1.0
            aug[gp + 2, la:la + nq] = fix_hi[qi].astype(np.float32)
            aug[gp + 3, la:la + nq] = fix_lo[qi].astype(np.float32)
            aug[gp + 0, ra:ra + ns] = cpos_hi[si].astype(np.float32)
            aug[gp + 1, ra:ra + ns] = cpos_lo[si].astype(np.float32)
            aug[gp + 0, ra + ns:ra + W] = 2000.0
            aug[gp + 0, ra + W:ra + NCb] = cagg_hi.astype(np.float32)
            aug[gp + 1, ra + W:ra + NCb] = cagg_lo.astype(np.float32)
            aug[gp + 2, ra + W + cls] = 1.0
            aug[gp + 3, ra + W + cls] = 1.0
            p, s = b // 2, b % 2
            P = pairs[p]
            for h in range(2):
                rows = slice(h * 128, (h + 1) * 128)
                P[:, s * 2 * QC + h * QC:s * 2 * QC + h * QC + nq] = xq_r8[qi].T[rows]
                rb = 4 * QC + s * 2 * NCb + h * NCb
                P[:, rb:rb + ns] = xs_r8[si].T[rows]
                P[:, rb + W:rb + NCb] = agg_r8.T[rows]
            core_meta.append((cls, qi))
        im = {
            "aug": aug.astype(bf16),
            "in01": np.concatenate([pairs[0], pairs[1]], axis=1),
            "in23": np.concatenate([pairs[2], pairs[3]], axis=1),
        }
        in_maps.append(im)
        meta.append(core_meta)

    host = {
        "SHP": SHP, "SA": SA,
        "tdev": np.exp(-((np.einsum('qd,qd->q',
                                    xq_r8.astype(np.float32),
                                    xs_r8[pos].astype(np.float32))
                          ).astype(np.float32).astype(np.float64)
                         + cpos_dev[pos])),
    }
    return QC, WS, in_maps, meta, host, Nq


def _reduce_host(results, meta, host, Nq):
    SHP, SA = host["SHP"], host["SA"]
    total = 0.0
    for k in range(NCORES):
        o = np.asarray(results[k]["out"], np.float64)
        for b, (cls, qi) in enumerate(meta[k]):
            n = len(qi)
            if n == 0:
                continue
            scol = b if b < 4 else 4 + b
            S = o[0:n, scol]
            Sn = o[0:n, scol + 4]
            S_excl = np.maximum(S - host["tdev"][qi], S * 1e-7)
            pos_v = np.log(S_excl) - SHP
            neg_v = np.log(Sn) - SA
            total += (neg_v - pos_v).sum()
    return np.array(total / Nq, dtype=np.float32)


def _run(xq, yq, xs, ys, pos, trace=False, tmpdir=None):
    from concourse import bass_utils

    xq = np.ascontiguousarray(np.asarray(xq, np.float32))
    xs = np.ascontiguousarray(np.asarray(xs, np.float32))
    yq = np.asarray(yq).astype(np.int64)
    ys = np.asarray(ys).astype(np.int64)
    pos = np.asarray(pos).astype(np.int64)

    QC, WS, in_maps, meta, host, Nq = _prepare(xq, yq, xs, ys, pos)
    key = (QC, tuple(WS))
    if key not in _PROGRAM_CACHE:
        _PROGRAM_CACHE[key] = _build_program(QC, WS)
    nc = _PROGRAM_CACHE[key]

    kw = {}
    if trace:
        kw = dict(trace=True, tmpdir=tmpdir)
    res = bass_utils.run_bass_kernel_spmd(
        nc, in_maps, core_ids=list(range(NCORES)), **kw
    )
    return _reduce_host(res.results, meta, host, Nq), res


def kernel(xq, yq, xs, ys, pos):
    loss, _ = _run(xq, yq, xs, ys, pos, trace=False)
    return loss


# revision 6
# speedup vs baseline: 1.8183x; 1.1218x over previous
"""Trainium2 Bass kernel for nn_KMLoss (segment_reduce proto-network loss).

Math (exact decomposition of the reference):
  L[q,s] = 0.5*|xq_q - xs_s|^2, logit = -L.  All device values drop the
  per-query term 0.5|xq|^2 (it cancels in neg-pos) and are shifted by global
  constants so fp32 exp never over/underflows:
    pos path:  P[q,j] = (0.5|xs_j|^2 - SHP) - xq.xs_j      (same-class j)
    neg path:  A[q,c] = (0.5*S2_c/cnt_c - SA) - xq.(T_c/cnt_c)
  The own-class column of A gets a per-query correction `fix` (removes the
  self entry and applies the -INF mask + cnt-1 denominator), folded into the
  matmul as an extra contraction row times an indicator column.  The device
  returns per-row exp-sums only; host takes logs, subtracts the (fp8-identical
  host-simulated) self column from the pos sum, and averages.

Per core: 8 class-blocks (core k owns classes ranked {b*8+k} by size).  Block
b owns PSUM bank b: a K=4 bf16 aug matmul (column constants + fix row; 4-way
row-tiled, start=True, runs while inputs stream) + two K=128 fp8 dot matmuls.
3 input DMAs (aug + two merged pair loads, one issued from the ACT ring to
parallelize HWDGE descriptor generation), dummy matmuls to lift the PE HAM
throttle during the load, chunked exp (ACT) + sum-reduce (DVE) sized to keep
the tail short (last block's sums via the ACT accumulator), 2 output DMAs.
"""

import sys

import numpy as np

sys.path.insert(0, "/opt/trn_rl_repo")

NCORES = 8
C = 64
CPB = C // NCORES
D = 256
INF = 1000.0

# exp/reduce chunking of the 8 blocks; last chunk's sums use ACT accum
CHUNKS = [(0, 2), (2, 4), (4, 6), (6, 7), (7, 8)]
PS_BANKS = [2, 2, 2, 1, 1]  # PSUM banks per chunk (sum = 8)
N_WARM_MM = 8

_PROGRAM_CACHE = {}


def _round_up(x, m):
    return -(-x // m) * m


def _build_program(QC, WS):
    import concourse.bacc as bacc
    import concourse.tile as tile
    from concourse import mybir

    dt = mybir.dt
    Alu = mybir.AluOpType
    Act = mybir.ActivationFunctionType
    f8 = dt.float8e4
    NC = [w + C for w in WS]

    nc = bacc.Bacc(
        "TRN2",
        target_bir_lowering=False,
        debug=False,
        enable_asserts=False,
        num_devices=NCORES,
    )

    # aug: row-group g in {0,1} (partitions 32g..32g+3) holds blocks with
    # b%2==g: [lhs x4 | rhs x4], zero-padded to uniform AUGW cols
    grp = [[b for b in range(8) if b % 2 == g] for g in range(2)]
    aug_cols = [4 * QC + sum(NC[b] for b in g) for g in grp]
    AUGW = max(aug_cols)
    aug_d = nc.dram_tensor("aug", [36, AUGW], dt.bfloat16, kind="ExternalInput").ap()
    pair_cols = [4 * QC + 2 * NC[2 * p] + 2 * NC[2 * p + 1] for p in range(4)]
    pair_d = [nc.dram_tensor(f"pair{p}", [128, pair_cols[p]], f8,
                             kind="ExternalInput").ap() for p in range(4)]
    out_d = nc.dram_tensor("out", [QC, 16], dt.float32, kind="ExternalOutput").ap()

    with tile.TileContext(nc) as tc:
        with (
            tc.tile_pool(name="io", bufs=1) as io,
            tc.tile_pool(name="pp", bufs=1, space="PSUM") as pp,
        ):
            warm_i = io.tile([128, 16], dt.float32)
            warm_o = io.tile([128, 16], dt.float32)
            w_s = io.tile([128, 128], dt.bfloat16)
            t_aug = io.tile([128, AUGW], dt.bfloat16)
            t_pair = [io.tile([128, pair_cols[p]], f8, name=f"t_pair{p}")
                      for p in range(4)]
            e_t = [io.tile([128, (hi - lo) * NC[lo]], dt.float32, name=f"e_t{lo}")
                   for lo, hi in CHUNKS]
            out0 = io.tile([128, 8], dt.float32)
            out1 = io.tile([128, 8], dt.float32)
            ps_t = [pp.tile([128, 512 * nb], dt.float32, name=f"ps_c{i}")
                    for i, nb in enumerate(PS_BANKS)]

            # scratch init on gpsimd (keeps DVE free); exp-table warm on ACT
            nc.gpsimd.memset(warm_i[:, :], 0.0)
            nc.gpsimd.memset(w_s[:, :], 0.0)
            nc.scalar.activation(warm_o[:, :], warm_i[:, :], Act.Exp)

            # input DMAs split across the two HWDGE rings so descriptor
            # generation runs in parallel and early pairs land first
            nc.sync.dma_start(out=t_aug[0:36, :], in_=aug_d)
            nc.sync.dma_start(out=t_pair[0], in_=pair_d[0])
            nc.scalar.dma_start(out=t_pair[2], in_=pair_d[2])
            nc.sync.dma_start(out=t_pair[1], in_=pair_d[1])
            nc.scalar.dma_start(out=t_pair[3], in_=pair_d[3])

            # dummy matmuls: lift the PE HAM clock gate while inputs stream
            for _ in range(N_WARM_MM):
                nc.tensor.matmul(
                    ps_t[0][0:128, 0:128], w_s[:, 0:128], w_s[:, 0:128],
                    start=True, stop=True,
                )

            def chunk_of(b):
                for ci, (lo, hi) in enumerate(CHUNKS):
                    if lo <= b < hi:
                        return ci, b - lo
                raise AssertionError

            def bank(b):
                ci, off = chunk_of(b)
                return ps_t[ci][0:QC, off * 512:off * 512 + NC[b]]

            # aug matmuls open each bank's accumulation group (2-way row tiled)
            for b in range(8):
                g = b % 2
                gi = grp[g].index(b)
                la = gi * QC
                ra = 4 * QC + sum(NC[x] for x in grp[g][:gi])
                nc.tensor.matmul(
                    bank(b),
                    t_aug[32 * g:32 * g + 4, la:la + QC],
                    t_aug[32 * g:32 * g + 4, ra:ra + NC[b]],
                    start=True,
                    stop=False,
                    tile_position=(32 * g, 0),
                )

            def emit_dots(b):
                p, s = b // 2, b % 2
                t_in = t_pair[p]
                rbase = 4 * QC + s * 2 * NC[b]
                for h in range(2):
                    nc.tensor.matmul(
                        bank(b),
                        t_in[:, s * 2 * QC + h * QC:s * 2 * QC + (h + 1) * QC],
                        t_in[:, rbase + h * NC[b]:rbase + (h + 1) * NC[b]],
                        start=False,
                        stop=(h == 1),
                    )

            def emit_chunk(ci):
                lo, hi = CHUNKS[ci]
                n = hi - lo
                W, NCb = WS[lo], NC[lo]
                ps = ps_t[ci]
                if n > 1:
                    src = ps[0:QC, 0:n * 512].rearrange(
                        "p (g c) -> p g c", g=n)[:, :, 0:NCb]
                    dst = e_t[ci][0:QC, :].rearrange("p (g c) -> p g c", c=NCb)
                else:
                    src = ps[0:QC, 0:NCb]
                    dst = e_t[ci][0:QC, :]
                ot = out0 if lo < 4 else out1
                oc = lo % 4
                if ci == len(CHUNKS) - 1:
                    # single block: exp pos / agg separately, sums via ACT accum
                    nc.scalar.activation(
                        e_t[ci][0:QC, 0:W], ps[0:QC, 0:W], Act.Exp, scale=-1.0,
                        accum_out=ot[0:QC, oc:oc + 1],
                    )
                    nc.scalar.activation(
                        e_t[ci][0:QC, W:NCb], ps[0:QC, W:NCb], Act.Exp, scale=-1.0,
                        accum_out=ot[0:QC, 4 + oc:4 + oc + 1],
                    )
                    return
                nc.scalar.activation(dst, src, Act.Exp, scale=-1.0)
                ev = e_t[ci][0:QC, :].rearrange("p (g c) -> p g c", c=NCb)
                nc.vector.tensor_reduce(
                    out=ot[0:QC, oc:oc + n], in_=ev[:, :, 0:W],
                    axis=mybir.AxisListType.X, op=Alu.add,
                )
                nc.vector.tensor_reduce(
                    out=ot[0:QC, 4 + oc:4 + oc + n], in_=ev[:, :, W:NCb],
                    axis=mybir.AxisListType.X, op=Alu.add,
                )

            for b in range(2):
                emit_dots(b)
            emit_chunk(0)
            for b in range(2, 4):
                emit_dots(b)
            emit_chunk(1)
            nc.sync.dma_start(out=out_d[:, 0:8], in_=out0[0:QC, :])
            for b in range(4, 6):
                emit_dots(b)
            emit_chunk(2)
            emit_dots(6)
            emit_chunk(3)
            emit_dots(7)
            emit_chunk(4)
            nc.sync.dma_start(out=out_d[:, 8:16], in_=out1[0:QC, :])

    nc.compile()
    return nc


def _prepare(xq, yq, xs, ys, pos):
    """Host-side prep: class stats, shifts, per-core packed arrays."""
    import ml_dtypes

    bf16 = ml_dtypes.bfloat16
    f8 = ml_dtypes.float8_e4m3
    Nq = xq.shape[0]
    xq64 = xq.astype(np.float64)
    xs64 = xs.astype(np.float64)

    cnt = np.bincount(ys, minlength=C).astype(np.float64)
    assert (cnt > 1).all(), "singleton/empty classes unsupported by this kernel"
    T_c = np.zeros((C, D))
    np.add.at(T_c, ys, xs64)
    S2_c = np.zeros(C)
    np.add.at(S2_c, ys, (xs64 ** 2).sum(-1))
    xs2h = 0.5 * (xs64 ** 2).sum(-1)
    xq2h = 0.5 * (xq64 ** 2).sum(-1)
    agg_scaled = T_c / cnt[:, None]

    A_exact = 0.5 * S2_c[None, :] / cnt[None, :] - xq.astype(np.float32) @ \
        agg_scaled.T.astype(np.float32)
    SA = float(np.round(np.median(A_exact)))
    assert np.abs(A_exact - SA).max() < 70, "neg-path shift margin exceeded"

    sidx = [np.where(ys == c)[0] for c in range(C)]
    qidx = [np.where(yq == c)[0] for c in range(C)]
    xsf = xs.astype(np.float32)
    xqf = xq.astype(np.float32)
    pm_lo, pm_hi = 1e30, -1e30
    for c in range(C):
        if len(qidx[c]) == 0 or len(sidx[c]) == 0:
            continue
        P = xs2h[sidx[c]][None, :].astype(np.float32) - xqf[qidx[c]] @ xsf[sidx[c]].T
        m = P.min(axis=1)
        pm_lo = min(pm_lo, float(m.min()))
        pm_hi = max(pm_hi, float(m.max()))
    assert pm_hi - pm_lo < 150, "pos-path shift window too wide"
    SHP = float(np.round((pm_lo + pm_hi) / 2))

    H = 0.5 * S2_c[yq] - np.einsum('qd,qd->q', xq64, T_c[yq])
    Lself_t = xs2h[pos] - np.einsum('qd,qd->q', xq64, xs64[pos])
    adj = cnt[yq] - 1.0
    fix = xq2h / adj + H / (cnt[yq] * adj) + (INF - (Lself_t + xq2h)) / adj

    def hilo(v):
        hi = v.astype(bf16).astype(np.float64)
        lo = (v - hi).astype(bf16)
        return hi.astype(bf16), lo

    fix_hi, fix_lo = hilo(fix)
    cpos_hi, cpos_lo = hilo(xs2h - SHP)
    cagg_hi, cagg_lo = hilo(0.5 * S2_c / cnt - SA)
    cpos_dev = cpos_hi.astype(np.float64) + cpos_lo.astype(np.float64)

    order = np.argsort(-cnt, kind='stable')
    QC = _round_up(max(max(len(q) for q in qidx), 1), 8)
    WS = [0] * 8
    for b in range(8):
        WS[b] = _round_up(max(len(sidx[order[b * 8 + k]]) for k in range(NCORES)), 16)
    for lo, hi in CHUNKS:
        w = max(WS[lo:hi])
        for b in range(lo, hi):
            WS[b] = w
    NC = [w + C for w in WS]
    assert max(NC) <= 512 and QC <= 128

    xq_r8 = (-xqf).astype(f8)
    xs_r8 = xsf.astype(f8)
    agg_r8 = agg_scaled.astype(np.float32).astype(f8)

    grp = [[b for b in range(8) if b % 2 == g] for g in range(2)]
    aug_cols = [4 * QC + sum(NC[b] for b in g) for g in grp]
    AUGW = max(aug_cols)
    pair_cols = [4 * QC + 2 * NC[2 * p] + 2 * NC[2 * p + 1] for p in range(4)]

    in_maps = []
    meta = []
    for k in range(NCORES):
        aug = np.zeros((36, AUGW), np.float32)
        pairs = [np.zeros((128, pair_cols[p]), f8) for p in range(4)]
        core_meta = []
        for b in range(8):
            cls = int(order[b * 8 + k])
            qi = qidx[cls]
            si = sidx[cls]
            nq, ns = len(qi), len(si)
            W, NCb = WS[b], NC[b]
            g = b % 2
            gi = grp[g].index(b)
            la = gi * QC
            ra = 4 * QC + sum(NC[x] for x in grp[g][:gi])
            gp = 32 * g
            aug[gp + 0, la:la + nq] = 1.0
            aug[gp + 1, la:la + nq] = 1.0
            aug[gp + 2, la:la + nq] = fix_hi[qi].astype(np.float32)
            aug[gp + 3, la:la + nq] = fix_lo[qi].astype(np.float32)
            aug[gp + 0, ra:ra + ns] = cpos_hi[si].astype(np.float32)
            aug[gp + 1, ra:ra + ns] = cpos_lo[si].astype(np.float32)
            aug[gp + 0, ra + ns:ra + W] = 2000.0
            aug[gp + 0, ra + W:ra + NCb] = cagg_hi.astype(np.float32)
            aug[gp + 1, ra + W:ra + NCb] = cagg_lo.astype(np.float32)
            aug[gp + 2, ra + W + cls] = 1.0
            aug[gp + 3, ra + W + cls] = 1.0
            p, s = b // 2, b % 2
            P = pairs[p]
            for h in range(2):
                rows = slice(h * 128, (h + 1) * 128)
                P[:, s * 2 * QC + h * QC:s * 2 * QC + h * QC + nq] = xq_r8[qi].T[rows]
                rb = 4 * QC + s * 2 * NCb + h * NCb
                P[:, rb:rb + ns] = xs_r8[si].T[rows]
                P[:, rb + W:rb + NCb] = agg_r8.T[rows]
            core_meta.append((cls, qi))
        im = {"aug": aug.astype(bf16)}
        for p in range(4):
            im[f"pair{p}"] = pairs[p]
        in_maps.append(im)
        meta.append(core_meta)

    host = {
        "SHP": SHP, "SA": SA,
        "tdev": np.exp(-((np.einsum('qd,qd->q',
                                    xq_r8.astype(np.float32),
                                    xs_r8[pos].astype(np.float32))
                          ).astype(np.float32).astype(np.float64)
                         + cpos_dev[pos])),
    }
    return QC, WS, in_maps, meta, host, Nq


def _reduce_host(results, meta, host, Nq):
    SHP, SA = host["SHP"], host["SA"]
    total = 0.0
    for k in range(NCORES):
        o = np.asarray(results[k]["out"], np.float64)
        for b, (cls, qi) in enumerate(meta[k]):
            n = len(qi)
            if n == 0:
                continue
            scol = b if b < 4 else 4 + b
            S = o[0:n, scol]
            Sn = o[0:n, scol + 4]
            S_excl = np.maximum(S - host["tdev"][qi], S * 1e-7)
            pos_v = np.log(S_excl) - SHP
            neg_v = np.log(Sn) - SA
            total += (neg_v - pos_v).sum()
    return np.array(total / Nq, dtype=np.float32)


def _run(xq, yq, xs, ys, pos, trace=False, tmpdir=None):
    from concourse import bass_utils

    xq = np.ascontiguousarray(np.asarray(xq, np.float32))
    xs = np.ascontiguousarray(np.asarray(xs, np.float32))
    yq = np.asarray(yq).astype(np.int64)
    ys = np.asarray(ys).astype(np.int64)
    pos = np.asarray(pos).astype(np.int64)

    QC, WS, in_maps, meta, host, Nq = _prepare(xq, yq, xs, ys, pos)
    key = (QC, tuple(WS))
    if key not in _PROGRAM_CACHE:
        _PROGRAM_CACHE[key] = _build_program(QC, WS)
    nc = _PROGRAM_CACHE[key]

    kw = {}
    if trace:
        kw = dict(trace=True, tmpdir=tmpdir)
    res = bass_utils.run_bass_kernel_spmd(
        nc, in_maps, core_ids=list(range(NCORES)), **kw
    )
    return _reduce_host(res.results, meta, host, Nq), res


def kernel(xq, yq, xs, ys, pos):
    loss, _ = _run(xq, yq, xs, ys, pos, trace=False)
    return loss
